# revision 39
# baseline (speedup 1.0000x reference)
"""ArcFace loss (B=8192, D=512, C=500000) on 8 TRN2 NeuronCores.

v8 strategy - the device kernel is reduced to the one irreducible piece of
work: the B x B cosine matmul and the row-wise sum-of-exp.  Everything
else (per-row scalars, O(B*D) vector math) runs on the host:
  - Host gathers centers = W[labels], L2-normalizes both x and the
    centers, pre-scales by 16 and casts to fp8e4 (the matmul then yields
    256*cos, and the device exp uses the constant scale S/256); it also
    computes the exact diagonal cosine t_i = xn_i . cn_i, the margin term
    t' = cos(arccos(t)+M), the sum-exp diagonal correction, and assembles
    the final label-smoothed loss from the device row-sums.
  - Device (row-sharded, core k owns batch rows [k*1024, (k+1)*1024)):
    stream all 8192 normalized centers (fp8, replicated 4MB) against the
    core's own 1024 x-rows (stationary fp8 SwInterleave blocks); 256
    DoubleRowSwInterleave matmuls at the PE's 215ns/512-col streaming
    rate with LDWEIGHTS fully hidden.  No collective, no device
    prefix/tail: each core DMAs out 64 partial sums, host combines.
  - The exp+row-sum consumers are the bottleneck, so each 2048-column
    psum block is split across TWO engines: 1536 columns to ScalarE
    (exact Exp, in-place on psum, accum_out row-sum; ~1.66us, under the
    PE's 1.72us/block) and 512 columns to DVE via a Schraudolph bit-trick
    exp (uint16(A*x+B) bitcast to bf16, ~2-4%/element error that washes
    out of the 8192-term sums; loss rel err stays ~1.2e-4), summed by a
    deferred accumulate-copy.  psd bufs=1 caps the D-stream run-ahead so
    PE-filler work survives to the end of the stream; the D matmuls are
    emitted first in each block to hide the exp chain's psum-recycle
    latency; 14 dummy warm-up matmuls bring the PE out of its low
    p-state while the first DMA pieces land.
"""

import sys

if "/opt/trn_rl_repo" not in sys.path:
    sys.path.insert(0, "/opt/trn_rl_repo")

import math

import numpy as np
import ml_dtypes

import concourse.bacc as bacc
import concourse.tile as tile
from concourse import mybir
from concourse.bass_utils import run_bass_kernel_spmd

F32 = mybir.dt.float32
BF16 = mybir.dt.bfloat16
FP8 = mybir.dt.float8e4
I32 = mybir.dt.int32
U16 = mybir.dt.uint16
P = 128

# problem constants (hardcoded; kernel.py must be self-contained)
B, D, C = 8192, 512, 500000
NCORES = 8
MARGIN, S_SCALE, EPS = 0.5, 64.0, 0.1
GAM = 16.0                       # fp8 pre-scale on xn and cn
EXP_SCALE = S_SCALE / (GAM * GAM)

BL = B // NCORES                 # 1024 own rows per core
NM = BL // P                     # 8 own row tiles
KC = D // P                      # 4 contraction chunks of 128
NKG = KC // 2                    # 2 double-row passes
NC_CH = 4                        # column chunks per row tile
CW = B // NC_CH                  # 2048 columns per chunk (4 psum banks)
NH = CW // 512                   # 4 matmuls of 512 per (chunk, kg)

# Schraudolph fast-exp constants for the DVE offload path, in bf16 bit
# space: exp(EXP_SCALE*x) ~= bitcast_bf16(uint16(A*x + B)); ~2-4%
# per-element error that washes out of the 8192-term row sums (validated:
# loss rel err stays ~1.2e-4 even with every term approximated).  The u16
# output makes the follow-up accumulate-copy all-2-byte, enabling the DVE
# 2x fast path.
SCHRAUD_A = (2.0**7 / math.log(2.0)) * EXP_SCALE  # 2^7/ln(2) * exp scale
SCHRAUD_B = 16256.0 - 486411.0 / 65536.0          # 127*2^7 - bias corr.


def build_nc():
    nc = bacc.Bacc(
        "TRN2",
        target_bir_lowering=False,
        debug=False,
        enable_asserts=False,
        num_devices=NCORES,
    )
    xw_ext = nc.dram_tensor("xw8", [P, NM * NKG * 2 * P], FP8, kind="ExternalInput")
    cn_ext = nc.dram_tensor("cnt8", [P, NC_CH * KC * CW], FP8, kind="ExternalInput")
    out_ext = nc.dram_tensor("sout", [P, NM * NC_CH * 2], F32, kind="ExternalOutput")

    with tile.TileContext(nc) as tc:
        with (
            tc.tile_pool(name="const", bufs=1) as const,
            tc.tile_pool(name="appr", bufs=3) as appr,
            tc.tile_pool(name="psa", bufs=2, space="PSUM") as psa,
            tc.tile_pool(name="psd", bufs=1, space="PSUM") as psd,
        ):
            xw = const.tile([P, NM * NKG * 2 * P], FP8, name="xw")
            cnt = const.tile([P, NC_CH * KC * CW], FP8, name="cnt")
            seacc = const.tile([P, NM * NC_CH * 2], F32, name="seacc")

            cnt3 = cnt[:].rearrange("p (k n) -> p k n", n=CW)
            cn_ext3 = cn_ext[:, :].rearrange("p (k n) -> p k n", n=CW)

            # lead-in: contiguous pieces only (strided sub-chunks transfer
            # at a fraction of the 360 GB/s aggregate rate), in exact matmul
            # consumption order: the first row tile's stationary blocks,
            # then chunk 0's four K-slices, the remaining stationaries, and
            # the other column chunks as whole 1 MB pieces.
            nc.sync.dma_start(out=xw[:, : 2 * 2 * P], in_=xw_ext[:, : 2 * 2 * P])
            nc.sync.dma_start(out=cnt3[:, 0:2, :], in_=cn_ext3[:, 0:2, :])
            nc.sync.dma_start(out=cnt3[:, 2:4, :], in_=cn_ext3[:, 2:4, :])
            nc.sync.dma_start(out=xw[:, 2 * 2 * P :], in_=xw_ext[:, 2 * 2 * P :])
            for c in range(1, NC_CH):
                nc.sync.dma_start(
                    out=cnt3[:, c * KC : (c + 1) * KC, :],
                    in_=cn_ext3[:, c * KC : (c + 1) * KC, :],
                )

            pending_pass2 = []
            pending_dma = []

            def flush_pass2():
                while pending_pass2:
                    pending_pass2.pop(0)()

            def flush_dma():
                while pending_dma:
                    pending_dma.pop(0)()

            AW = 1536  # ScalarE sub-chunk (3 psum banks)

            # PE warm-up: ~3us of dummy matmuls over a memset scratch region
            # while the first DMA pieces land, so the tensor engine reaches
            # its full p-state (2.4GHz, reached after 3us of continuous
            # work) before the real stream starts instead of running the
            # first blocks at 1.2GHz
            warm = const.tile([P, 2 * P + 2 * 512], FP8, name="warm")
            # split the scratch memset across two idle engines so the
            # warm-up matmuls can start ~1us earlier - the p-state ramp
            # needs continuous PE work, so every cycle before the first
            # DMA piece lands counts double
            nc.gpsimd.memset(warm[:, : 2 * P], 0)
            nc.vector.memset(warm[:, 2 * P :], 0)
            wrhs = warm[:, 2 * P :].rearrange("p (k n) -> p k n", n=512)
            wps = psa.tile([P, AW], F32, name="pa")
            for _ in range(7):
                nc.tensor.matmul(
                    out=wps[:, 0:512],
                    lhsT=warm[:, : 2 * P],
                    rhs=wrhs,
                    start=True,
                    stop=True,
                    perf_mode=mybir.MatmulPerfMode.DoubleRowSwInterleave,
                )

            # Each 2048-column block is split 1536 (ScalarE exp, 1664ns)
            # + 512 (DVE Schraudolph, ~1390ns for both passes): strict
            # consumer alternation on different engines keeps every
            # consumer under the PE's 1720ns per block, so the pipeline
            # is PE-paced instead of exp-paced.
            for c in range(NC_CH):
                for m in range(NM):
                    pa_t = psa.tile([P, AW], F32, name="pa")
                    pd_t = psd.tile([P, CW - AW], F32, name="pd")
                    slot = (m * NC_CH + c) * 2
                    # D sub-chunk FIRST in the stream: its two matmuls have
                    # long-satisfied deps, so they execute while the A
                    # chain's psum-recycle wait (exp + accumulator read +
                    # semaphore, ~400ns beyond the exp itself) resolves -
                    # hiding that latency behind useful PE work
                    for kg in range(NKG):
                        wo = (m * NKG + kg) * 2 * P
                        h = AW // 512
                        nc.tensor.matmul(
                            out=pd_t[:],
                            lhsT=xw[:, wo : wo + 2 * P],
                            rhs=cnt3[
                                :,
                                c * KC + 2 * kg : c * KC + 2 * kg + 2,
                                h * 512 : (h + 1) * 512,
                            ],
                            start=(kg == 0),
                            stop=(kg == NKG - 1),
                            perf_mode=mybir.MatmulPerfMode.DoubleRowSwInterleave,
                        )
                    for kg in range(NKG):
                        wo = (m * NKG + kg) * 2 * P
                        for h in range(AW // 512):
                            nc.tensor.matmul(
                                out=pa_t[:, h * 512 : (h + 1) * 512],
                                lhsT=xw[:, wo : wo + 2 * P],
                                rhs=cnt3[
                                    :,
                                    c * KC + 2 * kg : c * KC + 2 * kg + 2,
                                    h * 512 : (h + 1) * 512,
                                ],
                                start=(kg == 0),
                                stop=(kg == NKG - 1),
                                perf_mode=mybir.MatmulPerfMode.DoubleRowSwInterleave,
                            )
                    # exp in place (PSUM out has lower access latency than
                    # SBUF and the exp values themselves are dead - only
                    # the accumulator row-sum is used)
                    nc.scalar.activation(
                        out=pa_t[:],
                        in_=pa_t[:],
                        func=mybir.ActivationFunctionType.Exp,
                        scale=EXP_SCALE,
                        accum_out=seacc[:, slot : slot + 1],
                    )
                    # Schraudolph fast-exp (mult+add, uint16 out) releases
                    # the psum; the row-sum accumulate-copy of the
                    # bitcast-bf16 values reads only SBUF and is deferred
                    # until after the next block's pass 1 so DVE (in-order)
                    # never delays a psum buffer behind accumulation work
                    scr = appr.tile([P, CW - AW], U16, name="scr")
                    nc.vector.tensor_scalar(
                        out=scr[:],
                        in0=pd_t[:],
                        scalar1=SCHRAUD_A,
                        scalar2=SCHRAUD_B,
                        op0=mybir.AluOpType.mult,
                        op1=mybir.AluOpType.add,
                    )
                    flush_pass2()

                    def pass2(scr=scr, slot=slot + 1):
                        scrb = appr.tile([P, CW - AW], BF16, name="scrb")
                        nc.vector.tensor_scalar(
                            out=scrb[:],
                            in0=scr[:].bitcast(BF16),
                            scalar1=1.0,
                            scalar2=0.0,
                            op0=mybir.AluOpType.mult,
                            op1=mybir.AluOpType.add,
                            accum_out=seacc[:, slot : slot + 1],
                        )

                    pending_pass2.append(pass2)
                    # a pending per-m output DMA is safe to emit once the
                    # flush above has emitted its final slot's write
                    flush_dma()
                    if c == NC_CH - 1:
                        # row tile m's eight accumulators are final once
                        # this block's deferred pass2 lands; queue its
                        # output DMA to be emitted after the next block's
                        # flush (Tile builds the dep graph in emission
                        # order, so emitting the read before the deferred
                        # write would see stale data)
                        def dma_m(m=m):
                            nc.sync.dma_start(
                                out=out_ext[:, m * 2 * NC_CH : (m + 1) * 2 * NC_CH],
                                in_=seacc[:, m * 2 * NC_CH : (m + 1) * 2 * NC_CH],
                            )

                        pending_dma.append(dma_m)

            flush_pass2()
            flush_dma()

    nc.compile()
    return nc


def _pack_stationary(xn8_rows):
    """[1024, 512] fp8 -> [128, NM*NKG*256] SwInterleave stationary blocks.

    Block (m, kg) at column offset (m*NKG+kg)*256 holds
    packed[p, 2*(127-r) + i] = xn8[m*128 + r, (2*kg+i)*128 + p].
    """
    a = xn8_rows.reshape(NM, P, NKG, 2, P)          # [m, r, kg, i, p]
    a = a.transpose(4, 0, 2, 1, 3)[:, :, :, ::-1, :]  # [p, m, kg, r(rev), i]
    return np.ascontiguousarray(a.reshape(P, NM * NKG * 2 * P))


def _pack_streaming(cn8):
    """[8192, 512] fp8 -> [128, NC_CH*KC*CW]: piece (c, kk) holds
    cn8.T[kk*128 + p, c*CW + n]."""
    a = cn8.T.reshape(KC, P, NC_CH, CW).transpose(1, 2, 0, 3)  # [p, c, kk, n]
    return np.ascontiguousarray(a.reshape(P, NC_CH * KC * CW))


def prepare(x, labels, W):
    """All host-side math: normalize, pack fp8 inputs, and return the
    per-row constants needed to assemble the loss from device row-sums."""
    x = np.asarray(x, dtype=np.float32)
    W = np.asarray(W, dtype=np.float32)
    labels = np.asarray(labels).astype(np.int64)

    centers = W[labels]                                  # [B, D]
    cn = centers / np.linalg.norm(centers, axis=1, keepdims=True)
    xn = x / np.maximum(np.linalg.norm(x, axis=1, keepdims=True), 1e-12)

    xn8 = (xn * GAM).astype(ml_dtypes.float8_e4m3)
    cn8 = (cn * GAM).astype(ml_dtypes.float8_e4m3)

    cnt = _pack_streaming(cn8)
    in_maps = []
    for k in range(NCORES):
        xw = _pack_stationary(xn8[k * BL : (k + 1) * BL])
        in_maps.append({"xw8": xw, "cnt8": cnt})

    # exact per-row scalars in f64
    xn64 = xn.astype(np.float64)
    cn64 = cn.astype(np.float64)
    t = np.clip(np.sum(xn64 * cn64, axis=1), -1.0, 1.0)
    tp = np.cos(np.arccos(t) + MARGIN)
    ecorr = np.exp(S_SCALE * tp) - np.exp(S_SCALE * t)
    rowlin = xn64 @ cn64.sum(axis=0) + (tp - t)          # sum_j cos'_ij
    return in_maps, t, tp, ecorr, rowlin


_compiled_nc = None


def get_compiled():
    global _compiled_nc
    if _compiled_nc is None:
        _compiled_nc = build_nc()
    return _compiled_nc


def run(x, labels, W, trace=False, trace_cores=None):
    nc = get_compiled()
    in_maps, t, tp, ecorr, rowlin = prepare(x, labels, W)
    res = run_bass_kernel_spmd(
        nc,
        in_maps,
        core_ids=list(range(NCORES)),
        trace=trace,
        trace_cores=trace_cores,
    )
    # sout[p, m*8 + 2*c + {0,1}] holds the exp/approx partial sums over
    # column block c for local row m*128 + p; sum the 8 partials per row
    # tile, then flatten [m, p] -> local rows
    rowsum = np.concatenate(
        [
            np.asarray(r["sout"], dtype=np.float64)
            .reshape(P, NM, 2 * NC_CH)
            .sum(axis=2)
            .T.reshape(BL)
            for r in res.results
        ]
    )
    lse = np.log(rowsum + ecorr)
    a1 = (1.0 - EPS) + EPS * B / C
    loss = np.mean(
        a1 * lse - (1.0 - EPS) * S_SCALE * tp - (EPS / C) * S_SCALE * rowlin
    )
    return np.float32(loss), res


def kernel(**inputs):
    loss, _ = run(inputs["x"], inputs["labels"], inputs["W"])
    return loss


# revision 40
# speedup vs baseline: 1.0460x; 1.0460x over previous
"""ArcFace loss (B=8192, D=512, C=500000) on 8 TRN2 NeuronCores.

v8 strategy - the device kernel is reduced to the one irreducible piece of
work: the B x B cosine matmul and the row-wise sum-of-exp.  Everything
else (per-row scalars, O(B*D) vector math) runs on the host:
  - Host gathers centers = W[labels], L2-normalizes both x and the
    centers, pre-scales by 16 and casts to fp8e4 (the matmul then yields
    256*cos, and the device exp uses the constant scale S/256); it also
    computes the exact diagonal cosine t_i = xn_i . cn_i, the margin term
    t' = cos(arccos(t)+M), the sum-exp diagonal correction, and assembles
    the final label-smoothed loss from the device row-sums.
  - Device (row-sharded, core k owns batch rows [k*1024, (k+1)*1024)):
    stream all 8192 normalized centers (fp8, replicated 4MB) against the
    core's own 1024 x-rows (stationary fp8 SwInterleave blocks); 256
    DoubleRowSwInterleave matmuls at the PE's 215ns/512-col streaming
    rate with LDWEIGHTS fully hidden.  No collective, no device
    prefix/tail: each core DMAs out 64 partial sums, host combines.
  - The exp+row-sum consumers are the bottleneck, so each 2048-column
    psum block is split across TWO engines: 1536 columns to ScalarE
    (exact Exp, in-place on psum, accum_out row-sum; ~1.66us, under the
    PE's 1.72us/block) and 512 columns to DVE via a Schraudolph bit-trick
    exp (uint16(A*x+B) bitcast to bf16, ~2-4%/element error that washes
    out of the 8192-term sums; loss rel err stays ~1.2e-4), summed by a
    deferred accumulate-copy.  psd bufs=1 caps the D-stream run-ahead so
    PE-filler work survives to the end of the stream; the D matmuls are
    emitted first in each block to hide the exp chain's psum-recycle
    latency; 14 dummy warm-up matmuls bring the PE out of its low
    p-state while the first DMA pieces land.
"""

import sys

if "/opt/trn_rl_repo" not in sys.path:
    sys.path.insert(0, "/opt/trn_rl_repo")

import math

import numpy as np
import ml_dtypes

import concourse.bacc as bacc
import concourse.tile as tile
from concourse import mybir
from concourse.bass_utils import run_bass_kernel_spmd

F32 = mybir.dt.float32
BF16 = mybir.dt.bfloat16
FP8 = mybir.dt.float8e4
I32 = mybir.dt.int32
U16 = mybir.dt.uint16
P = 128

# problem constants (hardcoded; kernel.py must be self-contained)
B, D, C = 8192, 512, 500000
NCORES = 8
MARGIN, S_SCALE, EPS = 0.5, 64.0, 0.1
GAM = 16.0                       # fp8 pre-scale on xn and cn
EXP_SCALE = S_SCALE / (GAM * GAM)

BL = B // NCORES                 # 1024 own rows per core
NM = BL // P                     # 8 own row tiles
KC = D // P                      # 4 contraction chunks of 128
NKG = KC // 2                    # 2 double-row passes
NC_CH = 4                        # column chunks per row tile
CW = B // NC_CH                  # 2048 columns per chunk (4 psum banks)
NH = CW // 512                   # 4 matmuls of 512 per (chunk, kg)

# Schraudolph fast-exp constants for the DVE offload path, in bf16 bit
# space: exp(EXP_SCALE*x) ~= bitcast_bf16(uint16(A*x + B)); ~2-4%
# per-element error that washes out of the 8192-term row sums (validated:
# loss rel err stays ~1.2e-4 even with every term approximated).  The u16
# output makes the follow-up accumulate-copy all-2-byte, enabling the DVE
# 2x fast path.
SCHRAUD_A = (2.0**7 / math.log(2.0)) * EXP_SCALE  # 2^7/ln(2) * exp scale
SCHRAUD_B = 16256.0 - 486411.0 / 65536.0          # 127*2^7 - bias corr.


def build_nc():
    nc = bacc.Bacc(
        "TRN2",
        target_bir_lowering=False,
        debug=False,
        enable_asserts=False,
        num_devices=NCORES,
    )
    xw_ext = nc.dram_tensor("xw8", [P, NM * NKG * 2 * P], FP8, kind="ExternalInput")
    cn_ext = nc.dram_tensor("cnt8", [P, NC_CH * KC * CW], FP8, kind="ExternalInput")
    out_ext = nc.dram_tensor("sout", [P, NM * NC_CH * 2], F32, kind="ExternalOutput")

    with tile.TileContext(nc) as tc:
        with (
            tc.tile_pool(name="const", bufs=1) as const,
            tc.tile_pool(name="appr", bufs=3) as appr,
            tc.tile_pool(name="psa", bufs=2, space="PSUM") as psa,
            tc.tile_pool(name="psd", bufs=1, space="PSUM") as psd,
        ):
            xw = const.tile([P, NM * NKG * 2 * P], FP8, name="xw")
            cnt = const.tile([P, NC_CH * KC * CW], FP8, name="cnt")
            seacc = const.tile([P, NM * NC_CH * 2], F32, name="seacc")

            cnt3 = cnt[:].rearrange("p (k n) -> p k n", n=CW)
            cn_ext3 = cn_ext[:, :].rearrange("p (k n) -> p k n", n=CW)

            # lead-in: contiguous pieces only (strided sub-chunks transfer
            # at a fraction of the 360 GB/s aggregate rate), in exact matmul
            # consumption order: the first row tile's stationary blocks,
            # then chunk 0's four K-slices, the remaining stationaries, and
            # the other column chunks as whole 1 MB pieces.
            nc.sync.dma_start(out=xw[:, : 2 * 2 * P], in_=xw_ext[:, : 2 * 2 * P])
            nc.sync.dma_start(out=cnt3[:, 0:2, :], in_=cn_ext3[:, 0:2, :])
            nc.sync.dma_start(out=cnt3[:, 2:4, :], in_=cn_ext3[:, 2:4, :])
            nc.sync.dma_start(out=xw[:, 2 * 2 * P :], in_=xw_ext[:, 2 * 2 * P :])
            for c in range(1, NC_CH):
                nc.sync.dma_start(
                    out=cnt3[:, c * KC : (c + 1) * KC, :],
                    in_=cn_ext3[:, c * KC : (c + 1) * KC, :],
                )

            pending_pass2 = []
            pending_dma = []

            def flush_pass2():
                while pending_pass2:
                    pending_pass2.pop(0)()

            def flush_dma():
                while pending_dma:
                    pending_dma.pop(0)()

            AW = 1536  # ScalarE sub-chunk (3 psum banks)

            # PE warm-up: ~3us of dummy matmuls over a memset scratch region
            # while the first DMA pieces land, so the tensor engine reaches
            # its full p-state (2.4GHz, reached after 3us of continuous
            # work) before the real stream starts instead of running the
            # first blocks at 1.2GHz
            warm = const.tile([P, 2 * P + 2 * 512], FP8, name="warm")
            nc.gpsimd.memset(warm[:], 0)
            wrhs = warm[:, 2 * P :].rearrange("p (k n) -> p k n", n=512)
            wps = psa.tile([P, AW], F32, name="pa")
            for _ in range(6):
                nc.tensor.matmul(
                    out=wps[:, 0:512],
                    lhsT=warm[:, : 2 * P],
                    rhs=wrhs,
                    start=True,
                    stop=True,
                    perf_mode=mybir.MatmulPerfMode.DoubleRowSwInterleave,
                )

            # Each 2048-column block is split 1536 (ScalarE exp, 1664ns)
            # + 512 (DVE Schraudolph, ~1390ns for both passes): strict
            # consumer alternation on different engines keeps every
            # consumer under the PE's 1720ns per block, so the pipeline
            # is PE-paced instead of exp-paced.
            for c in range(NC_CH):
                for m in range(NM):
                    pa_t = psa.tile([P, AW], F32, name="pa")
                    pd_t = psd.tile([P, CW - AW], F32, name="pd")
                    slot = (m * NC_CH + c) * 2
                    # D sub-chunk FIRST in the stream: its two matmuls have
                    # long-satisfied deps, so they execute while the A
                    # chain's psum-recycle wait (exp + accumulator read +
                    # semaphore, ~400ns beyond the exp itself) resolves -
                    # hiding that latency behind useful PE work
                    for kg in range(NKG):
                        wo = (m * NKG + kg) * 2 * P
                        h = AW // 512
                        nc.tensor.matmul(
                            out=pd_t[:],
                            lhsT=xw[:, wo : wo + 2 * P],
                            rhs=cnt3[
                                :,
                                c * KC + 2 * kg : c * KC + 2 * kg + 2,
                                h * 512 : (h + 1) * 512,
                            ],
                            start=(kg == 0),
                            stop=(kg == NKG - 1),
                            perf_mode=mybir.MatmulPerfMode.DoubleRowSwInterleave,
                        )
                    for kg in range(NKG):
                        wo = (m * NKG + kg) * 2 * P
                        for h in range(AW // 512):
                            nc.tensor.matmul(
                                out=pa_t[:, h * 512 : (h + 1) * 512],
                                lhsT=xw[:, wo : wo + 2 * P],
                                rhs=cnt3[
                                    :,
                                    c * KC + 2 * kg : c * KC + 2 * kg + 2,
                                    h * 512 : (h + 1) * 512,
                                ],
                                start=(kg == 0),
                                stop=(kg == NKG - 1),
                                perf_mode=mybir.MatmulPerfMode.DoubleRowSwInterleave,
                            )
                    # exp in place (PSUM out has lower access latency than
                    # SBUF and the exp values themselves are dead - only
                    # the accumulator row-sum is used)
                    nc.scalar.activation(
                        out=pa_t[:],
                        in_=pa_t[:],
                        func=mybir.ActivationFunctionType.Exp,
                        scale=EXP_SCALE,
                        accum_out=seacc[:, slot : slot + 1],
                    )
                    # Schraudolph fast-exp (mult+add, uint16 out) releases
                    # the psum; the row-sum accumulate-copy of the
                    # bitcast-bf16 values reads only SBUF and is deferred
                    # until after the next block's pass 1 so DVE (in-order)
                    # never delays a psum buffer behind accumulation work
                    scr = appr.tile([P, CW - AW], U16, name="scr")
                    nc.vector.tensor_scalar(
                        out=scr[:],
                        in0=pd_t[:],
                        scalar1=SCHRAUD_A,
                        scalar2=SCHRAUD_B,
                        op0=mybir.AluOpType.mult,
                        op1=mybir.AluOpType.add,
                    )
                    flush_pass2()

                    def pass2(scr=scr, slot=slot + 1):
                        scrb = appr.tile([P, CW - AW], BF16, name="scrb")
                        nc.vector.tensor_scalar(
                            out=scrb[:],
                            in0=scr[:].bitcast(BF16),
                            scalar1=1.0,
                            scalar2=0.0,
                            op0=mybir.AluOpType.mult,
                            op1=mybir.AluOpType.add,
                            accum_out=seacc[:, slot : slot + 1],
                        )

                    pending_pass2.append(pass2)
                    # a pending per-m output DMA is safe to emit once the
                    # flush above has emitted its final slot's write
                    flush_dma()
                    if c == NC_CH - 1:
                        # row tile m's eight accumulators are final once
                        # this block's deferred pass2 lands; queue its
                        # output DMA to be emitted after the next block's
                        # flush (Tile builds the dep graph in emission
                        # order, so emitting the read before the deferred
                        # write would see stale data)
                        def dma_m(m=m):
                            nc.sync.dma_start(
                                out=out_ext[:, m * 2 * NC_CH : (m + 1) * 2 * NC_CH],
                                in_=seacc[:, m * 2 * NC_CH : (m + 1) * 2 * NC_CH],
                            )

                        pending_dma.append(dma_m)

            flush_pass2()
            flush_dma()

    nc.compile()
    return nc


def _pack_stationary(xn8_rows):
    """[1024, 512] fp8 -> [128, NM*NKG*256] SwInterleave stationary blocks.

    Block (m, kg) at column offset (m*NKG+kg)*256 holds
    packed[p, 2*(127-r) + i] = xn8[m*128 + r, (2*kg+i)*128 + p].
    """
    a = xn8_rows.reshape(NM, P, NKG, 2, P)          # [m, r, kg, i, p]
    a = a.transpose(4, 0, 2, 1, 3)[:, :, :, ::-1, :]  # [p, m, kg, r(rev), i]
    return np.ascontiguousarray(a.reshape(P, NM * NKG * 2 * P))


def _pack_streaming(cn8):
    """[8192, 512] fp8 -> [128, NC_CH*KC*CW]: piece (c, kk) holds
    cn8.T[kk*128 + p, c*CW + n]."""
    a = cn8.T.reshape(KC, P, NC_CH, CW).transpose(1, 2, 0, 3)  # [p, c, kk, n]
    return np.ascontiguousarray(a.reshape(P, NC_CH * KC * CW))


def prepare(x, labels, W):
    """All host-side math: normalize, pack fp8 inputs, and return the
    per-row constants needed to assemble the loss from device row-sums."""
    x = np.asarray(x, dtype=np.float32)
    W = np.asarray(W, dtype=np.float32)
    labels = np.asarray(labels).astype(np.int64)

    centers = W[labels]                                  # [B, D]
    cn = centers / np.linalg.norm(centers, axis=1, keepdims=True)
    xn = x / np.maximum(np.linalg.norm(x, axis=1, keepdims=True), 1e-12)

    xn8 = (xn * GAM).astype(ml_dtypes.float8_e4m3)
    cn8 = (cn * GAM).astype(ml_dtypes.float8_e4m3)

    cnt = _pack_streaming(cn8)
    in_maps = []
    for k in range(NCORES):
        xw = _pack_stationary(xn8[k * BL : (k + 1) * BL])
        in_maps.append({"xw8": xw, "cnt8": cnt})

    # exact per-row scalars in f64
    xn64 = xn.astype(np.float64)
    cn64 = cn.astype(np.float64)
    t = np.clip(np.sum(xn64 * cn64, axis=1), -1.0, 1.0)
    tp = np.cos(np.arccos(t) + MARGIN)
    ecorr = np.exp(S_SCALE * tp) - np.exp(S_SCALE * t)
    rowlin = xn64 @ cn64.sum(axis=0) + (tp - t)          # sum_j cos'_ij
    return in_maps, t, tp, ecorr, rowlin


_compiled_nc = None


def get_compiled():
    global _compiled_nc
    if _compiled_nc is None:
        _compiled_nc = build_nc()
    return _compiled_nc


def run(x, labels, W, trace=False, trace_cores=None):
    nc = get_compiled()
    in_maps, t, tp, ecorr, rowlin = prepare(x, labels, W)
    res = run_bass_kernel_spmd(
        nc,
        in_maps,
        core_ids=list(range(NCORES)),
        trace=trace,
        trace_cores=trace_cores,
    )
    # sout[p, m*8 + 2*c + {0,1}] holds the exp/approx partial sums over
    # column block c for local row m*128 + p; sum the 8 partials per row
    # tile, then flatten [m, p] -> local rows
    rowsum = np.concatenate(
        [
            np.asarray(r["sout"], dtype=np.float64)
            .reshape(P, NM, 2 * NC_CH)
            .sum(axis=2)
            .T.reshape(BL)
            for r in res.results
        ]
    )
    lse = np.log(rowsum + ecorr)
    a1 = (1.0 - EPS) + EPS * B / C
    loss = np.mean(
        a1 * lse - (1.0 - EPS) * S_SCALE * tp - (EPS / C) * S_SCALE * rowlin
    )
    return np.float32(loss), res


def kernel(**inputs):
    loss, _ = run(inputs["x"], inputs["labels"], inputs["W"])
    return loss
